# revision 32
# baseline (speedup 1.0000x reference)
"""Trainium2 Bass kernel for LocalNodeAttentionMultiHeadSumV1 (v2).

Data-parallel over batch: 16 batches across 8 NeuronCores (2 each), no
collectives.  Columns = pixels (hw, t) with t innermost, tiled 128 at a time
on the partition dim; all matmuls bf16 with f32 PSUM accumulation.

Per-core pipeline per column tile (pixels on partitions):
  scoresT = xT @ A^T (+bias row)      A = keys @ Wq folded on host
  vT      = xT @ Wv^T                 x tile chunk is the stationary operand
  alpha   = mask ⊙ softmax_k(scoresT) (exp on ACT; normalize+mask on DVE;
            mask kills out-of-window temporal taps so the shifted reads
            below never mix adjacent hw groups)
  yT      = sum_d alpha_(d+3) ⊙ shift_d(vT)   (fat DVE tensor_tensor ops,
            partition-shifted vT reads, alpha broadcast over each head's
            128 channels via stride-0 APs)
  y_n     = PE-transpose(yT_n)  (8 transposes into one bf16 PSUM bank)
  zT      = sum_n y_n^T? -> matmul(lhsT=y_n, rhs=WoT_n): out[col, c]
  out     = zT + xT (residual from host-transposed x) -> DMA (contiguous)

reps live in a HW loop (For_i) so program size is constant in reps; the
reps-diff timing then isolates pure device execution.
"""

import numpy as np
import ml_dtypes

import concourse.bass as bass
import concourse.mybir as mybir
import concourse.tile as tile
from concourse import bacc
from concourse.bass_utils import run_bass_kernel_spmd

F32 = mybir.dt.float32
BF16 = mybir.dt.bfloat16
FP8 = mybir.dt.float8e4
WSCALE = 16.0                    # fp8 weight up-scaling (see host_prep)

B, C, T, H, W = 16, 1024, 32, 7, 7
HWP = H * W                      # 49
KW, NH, CI = 7, 8, 128
N_CORES = 8
BS = B // N_CORES                # 2 batches per core
COLS = HWP * T                   # 1568 columns per batch (hw-major, t-inner)
NCC = C // 128                   # 8 channel chunks
NT = 13                          # column tiles per batch: 12 x 128 + 1 x 32
TILE_COLS = [128] * 12 + [32]
TILE_OFF = [128 * i for i in range(12)] + [1536]

_CACHE = {}


def _build(reps: int = 1, hw_loop: bool = True):
    """Build + compile the per-core Bass program (same on all 8 cores)."""
    nc = bacc.Bacc("TRN2", target_bir_lowering=False, debug=False)

    xin = nc.dram_tensor("xin", [BS, NCC, 128, COLS], FP8, kind="ExternalInput")
    xtin = nc.dram_tensor("xtin", [BS, COLS, C], BF16, kind="ExternalInput")
    wvt = nc.dram_tensor("wvt", [NCC, 128, NH * CI], FP8, kind="ExternalInput")
    at = nc.dram_tensor("at", [NCC, 128, 64], FP8, kind="ExternalInput")
    sbrow = nc.dram_tensor("sbrow", [1, 64], BF16, kind="ExternalInput")
    wott = nc.dram_tensor("wott", [NH, 128, C], FP8, kind="ExternalInput")
    zcrow = nc.dram_tensor("zcrow", [1, C], BF16, kind="ExternalInput")
    emat = nc.dram_tensor("emat", [KW, 128, 128], BF16, kind="ExternalInput")
    onesr = nc.dram_tensor("onesr", [1, 128], BF16, kind="ExternalInput")
    ident = nc.dram_tensor("ident", [128, 128], BF16, kind="ExternalInput")
    out = nc.dram_tensor("out", [BS, COLS, C], F32, kind="ExternalOutput")

    MULT = mybir.AluOpType.mult
    ADD = mybir.AluOpType.add
    AX = mybir.AxisListType.X
    EXP = mybir.ActivationFunctionType.Exp
    RCP = mybir.ActivationFunctionType.Reciprocal

    with tile.TileContext(nc) as tc:
        with (
            tc.tile_pool(name="xp", bufs=1) as xp,
            tc.tile_pool(name="wp", bufs=1) as wp,
            tc.tile_pool(name="vsb", bufs=3) as vsb,
            tc.tile_pool(name="ssb", bufs=3) as ssb,
            tc.tile_pool(name="gsb", bufs=4) as gsb,
            tc.tile_pool(name="ysb", bufs=3) as ysb,
            tc.tile_pool(name="ytrp", bufs=3) as ytrp,
            tc.tile_pool(name="osb", bufs=3) as osb,
            tc.tile_pool(name="psv", bufs=1, space="PSUM") as psv,
            tc.tile_pool(name="pss", bufs=2, space="PSUM") as pss,
            tc.tile_pool(name="psy", bufs=1, space="PSUM") as psy,
            tc.tile_pool(name="pst", bufs=1, space="PSUM") as pst,
            tc.tile_pool(name="psz", bufs=1, space="PSUM") as psz,
        ):
            # ---- persistent weights/constants ----
            wvt_t = wp.tile([128, NCC, NH * CI], FP8, tag="wvt", name="wvt_t")
            at_t = [wp.tile([128, 64], FP8, tag=f"at{c}", name=f"at{c}")
                    for c in range(NCC)]
            wott_t = wp.tile([128, NH, C], FP8, tag="wott", name="wott_t")
            sb_t = wp.tile([1, 64], BF16, tag="sbrow", name="sbrow_t")
            zc_t = wp.tile([1, C], BF16, tag="zcrow", name="zcrow_t")
            ones_t = wp.tile([1, 128], BF16, tag="onesr", name="onesr_t")
            id_t = wp.tile([128, 128], BF16, tag="ident", name="ident_t")
            e_t = [wp.tile([128, 128], BF16, tag=f"em{k}", name=f"em{k}")
                   for k in range(KW)]
            for c in range(NCC):
                nc.sync.dma_start(wvt_t[:, c], wvt.ap()[c])
                nc.sync.dma_start(at_t[c][:], at.ap()[c])
            for n in range(NH):
                nc.sync.dma_start(wott_t[:, n], wott.ap()[n])
            nc.sync.dma_start(sb_t[:], sbrow.ap())
            nc.sync.dma_start(zc_t[:], zcrow.ap())
            nc.sync.dma_start(ones_t[:], onesr.ap())
            nc.sync.dma_start(id_t[:], ident.ap())
            for k in range(KW):
                nc.sync.dma_start(e_t[k][:], emat.ap()[k])

            # ---- x tiles (both batches resident, both layouts) ----
            xb_t = [xp.tile([128, NCC, COLS], FP8, tag=f"xb{b}", name=f"xb{b}")
                    for b in range(BS)]
            xt_t = [xp.tile([128, NT, C], BF16, tag=f"xt{b}", name=f"xt{b}")
                    for b in range(BS)]
            for b in range(BS):
                for c in range(NCC):
                    nc.sync.dma_start(xb_t[b][:, c], xin.ap()[b, c])
                for it, (ncols, c0) in enumerate(zip(TILE_COLS, TILE_OFF)):
                    nc.sync.dma_start(xt_t[b][:ncols, it, :],
                                      xtin.ap()[b, c0:c0 + ncols, :])

            # reps in a HW loop: program size is constant in reps.
            import contextlib
            loop_cm = tc.For_i(0, reps, 1) if hw_loop else contextlib.nullcontext()
            with loop_cm:
                for b in range(BS):
                    for it, (ncols, c0) in enumerate(zip(TILE_COLS, TILE_OFF)):
                        _emit_tile(
                            nc, b, it, ncols, c0, xb_t, xt_t, wvt_t, at_t,
                            wott_t, sb_t, zc_t, ones_t, id_t, e_t, out,
                            vsb, ssb, gsb, ysb, ytrp, osb,
                            psv, pss, psy, pst, psz,
                            MULT, ADD, AX, EXP, RCP,
                        )

    nc.compile()
    return nc


def _emit_tile(nc, b, it, ncols, c0, xb_t, xt_t, wvt_t, at_t, wott_t, sb_t,
               zc_t, ones_t, id_t, e_t, out, vsb, ssb, gsb, ysb, ytrp, osb,
               psv, pss, psy, pst, psz, MULT, ADD, AX, EXP, RCP):
    cs = slice(c0, c0 + ncols)

    # ---- v^T and scores^T matmuls (x tile chunk as stationary operand) ----
    vt_ps = psv.tile([128, NH * CI], mybir.dt.float32, tag="vtps", name="vt_ps")
    sa_ps = pss.tile([128, 128], mybir.dt.float32, tag="scash", name="sa_ps")
    sc_ps = sa_ps[:, 0:64]
    DR = mybir.MatmulPerfMode.DoubleRow
    for c in range(NCC):
        nc.tensor.matmul(sc_ps[:ncols], xb_t[b][:, c, cs], at_t[c],
                         start=(c == 0), stop=False)
    nc.tensor.matmul(sc_ps[:ncols], ones_t[:, :ncols], sb_t[:],
                     start=False, stop=True)
    for cc in range(0, NCC, 2):
        lhs3 = xb_t[b][:, cc:cc + 2, cs]
        first, last = cc == 0, cc == NCC - 2
        nc.tensor.matmul(vt_ps[:ncols, 0:512], lhs3,
                         wvt_t[:, cc:cc + 2, 0:512],
                         start=first, stop=last, perf_mode=DR)
        nc.tensor.matmul(vt_ps[:ncols, 512:1024], lhs3,
                         wvt_t[:, cc:cc + 2, 512:1024],
                         start=first, stop=last, perf_mode=DR)

    # ---- evict v (ACT, fp32->bf16), exp(scores) (ACT) ----
    vt_sb = vsb.tile([128, NH * CI], BF16, tag="vtsb", name="vt_sb")
    nc.scalar.copy(vt_sb[:ncols], vt_ps[:ncols])
    e_sb = ssb.tile([128, 64], mybir.dt.float32, tag="esb", name="e_sb")
    nc.scalar.activation(e_sb[:ncols], sc_ps[:ncols], EXP, scale=1.0 / WSCALE)

    # ---- softmax over k; alpha emitted bf16, k-major ----
    e3 = e_sb[:ncols].rearrange("p (n k) -> p n k", k=8)[:, :, 0:KW]
    ssum = ssb.tile([128, 8], mybir.dt.float32, tag="ssum", name="ssum")
    nc.vector.tensor_reduce(ssum[:ncols], e3, axis=AX, op=ADD)
    rec = ssb.tile([128, 8], mybir.dt.float32, tag="rec", name="rec")
    nc.vector.reciprocal_approx_fast(rec[:ncols], ssum[:ncols])
    am_sb = ssb.tile([128, 64], BF16, tag="amsb", name="am_sb")
    am3 = am_sb[:ncols].rearrange("p (k n) -> p n k", k=8)[:, :, 0:KW]
    rec3 = rec[:ncols].unsqueeze(-1).broadcast_to((ncols, 8, KW))
    nc.vector.tensor_tensor(am3, e3, rec3, op=MULT)

    # ---- pre-shift alpha: ash_k[col] = alpha_k[col - d_k] via E_{6-k} ----
    # (shares the sc_ps PSUM slot; both are [128, 64] f32 and sequential)
    ash_ps = sa_ps[:, 64:128]
    for k in range(KW):
        nc.tensor.matmul(ash_ps[:ncols, k * 8:(k + 1) * 8],
                         e_t[6 - k][:ncols, :ncols],
                         am_sb[:ncols, k * 8:(k + 1) * 8],
                         start=(k == 0), stop=(k == KW - 1))
    ash_sb = ssb.tile([128, 64], BF16, tag="ashsb", name="ash_sb")
    nc.scalar.copy(ash_sb[:ncols], ash_ps[:ncols])

    # ---- windowed mix: yT = sum_k E_k^T @ (ash_k ⊙ vT) ----
    # g_k = ash_k ⊙ vT (fat DVE op, alpha broadcast over each head's Ci);
    # E_k is the group-masked temporal shift matrix (window mask folded in).
    yt_ps = psy.tile([128, NH * CI], mybir.dt.float32, tag="ytps", name="yt_ps")
    vt3 = vt_sb[:ncols].rearrange("p (n i) -> p n i", n=NH)
    for k in range(KW):
        g = gsb.tile([128, NH * CI], BF16, tag="g", name="g")
        amb = ash_sb[:ncols, k * 8:(k + 1) * 8].unsqueeze(-1).broadcast_to(
            (ncols, NH, CI))
        g3 = g[:ncols].rearrange("p (n i) -> p n i", n=NH)
        eng = nc.gpsimd if k in (0, 3, 6) else nc.vector
        eng.tensor_tensor(g3, vt3, amb, op=MULT)
        nc.tensor.matmul(yt_ps[:ncols, 0:512], e_t[k][:ncols, :ncols],
                         g[:ncols, 0:512], start=(k == 0), stop=(k == KW - 1))
        nc.tensor.matmul(yt_ps[:ncols, 512:1024], e_t[k][:ncols, :ncols],
                         g[:ncols, 512:1024], start=(k == 0), stop=(k == KW - 1))
    yt_sb = ysb.tile([128, NH * CI], BF16, tag="ytsb", name="yt_sb")
    nc.scalar.copy(yt_sb[:ncols], yt_ps[:ncols])

    # ---- per-head transpose back: 8 transposes into one bf16 PSUM bank ----
    tr_ps = pst.tile([128, NH * CI], BF16, tag="trps", name="tr_ps")
    for n in range(NH):
        nc.tensor.matmul(tr_ps[:, n * CI:n * CI + ncols],
                         yt_sb[:ncols, n * CI:(n + 1) * CI],
                         id_t[:ncols, :ncols],
                         is_transpose=True, start=(n == 0), stop=(n == NH - 1))
    ytr_sb = ytrp.tile([128, NH, CI], FP8, tag="ytrsb", name="ytr_sb")
    nc.scalar.copy(ytr_sb[:].rearrange("p n i -> p (n i)"), tr_ps[:])

    # ---- output projection in transposed form: zT[col, c] ----
    o_sb = osb.tile([128, C], mybir.dt.float32, tag="osb", name="o_sb")
    for h in range(2):
        hs = slice(h * 512, (h + 1) * 512)
        zt_ps = psz.tile([128, 512], mybir.dt.float32, tag="ztps", name="zt_ps")
        for nn in range(0, NH, 2):
            nc.tensor.matmul(zt_ps[:ncols], ytr_sb[:, nn:nn + 2, :ncols],
                             wott_t[:, nn:nn + 2, hs],
                             start=(nn == 0), stop=False, perf_mode=DR)
        nc.tensor.matmul(zt_ps[:ncols], ones_t[:, :ncols], zc_t[:, hs],
                         start=False, stop=True)
        nc.vector.scalar_tensor_tensor(o_sb[:ncols, hs], zt_ps[:ncols],
                                       1.0 / (WSCALE * WSCALE),
                                       xt_t[b][:ncols, it, hs],
                                       op0=MULT, op1=ADD)
    nc.sync.dma_start(out.ap()[b, c0:c0 + ncols, :], o_sb[:ncols])


def host_prep(x, nodes, Wq, bq, Wk, bk, Wv, bv, Wo, bo):
    """Fold biases, eliminate the Q projection, build device-layout arrays."""
    x = np.asarray(x, np.float32)
    keys = np.einsum("nij,nkj->nki", Wk, nodes) + bk[:, None, :]
    A = np.einsum("nki,nic->nkc", keys, Wq)                   # (N,K,C)
    sb = np.einsum("nki,ni->nk", keys, bq)                    # (N,K)
    zc = np.einsum("nci,ni->nc", Wo, bv).sum(0) / NH + bo.mean(0)

    bf = ml_dtypes.bfloat16
    f8 = mybir.dt.np(FP8)
    wvt = np.ascontiguousarray(
        Wv.reshape(NH * CI, C).T.reshape(NCC, 128, NH * CI) * WSCALE).astype(f8)
    A_pad = np.zeros((NH, 8, C), np.float32)
    A_pad[:, :KW] = A
    at = np.ascontiguousarray(
        A_pad.transpose(2, 0, 1).reshape(C, 64).reshape(NCC, 128, 64)
        * WSCALE).astype(f8)
    sbrow = (np.concatenate(
        [sb, np.zeros((NH, 1), np.float32)], 1).reshape(1, 64)
        * WSCALE).astype(bf)
    wott = np.ascontiguousarray(
        Wo.transpose(0, 2, 1) / NH * WSCALE).astype(f8)       # (NH, Ci, C)
    zcrow = (zc.reshape(1, C) * WSCALE * WSCALE).astype(bf)
    onesr = np.ones((1, 128), bf)
    ident = np.eye(128, dtype=np.float32).astype(bf)
    emat = np.zeros((KW, 128, 128), np.float32)
    for k in range(KW):
        d = k - 3
        for cp in range(128):
            col = cp - d
            if 0 <= col < 128 and col // T == cp // T:
                emat[k, cp, col] = 1.0
    emat = emat.astype(bf)

    # x natural: (core, b, cchunk, 128, hw*T) with t innermost
    xb = (x.reshape(B, NCC, 128, T, HWP).transpose(0, 1, 2, 4, 3)
          .reshape(B, NCC, 128, COLS)).astype(f8)
    # x transposed: (core, b, col=(hw,t), C)
    xt = np.ascontiguousarray(
        x.reshape(B, C, T, HWP).transpose(0, 3, 2, 1).reshape(B, COLS, C)
    ).astype(bf)
    shards = [
        (np.ascontiguousarray(xb[i * BS:(i + 1) * BS]),
         np.ascontiguousarray(xt[i * BS:(i + 1) * BS]))
        for i in range(N_CORES)
    ]
    shared = dict(wvt=wvt, at=at, sbrow=sbrow, wott=wott, zcrow=zcrow,
                  emat=emat, onesr=onesr, ident=ident)
    return shards, shared


def unprep_out(res_list):
    """(core results of (BS, COLS, C)) -> (B, C, T, H, W)"""
    full = np.concatenate(res_list, 0)
    return np.ascontiguousarray(
        full.reshape(B, HWP, T, C).transpose(0, 3, 2, 1).reshape(B, C, T, H, W))


def run_on_device(inputs, reps: int = 1):
    key = reps
    if key not in _CACHE:
        _CACHE[key] = _build(reps)
    nc = _CACHE[key]
    shards, shared = host_prep(**inputs)
    in_maps = [dict(xin=shards[i][0], xtin=shards[i][1], **shared)
               for i in range(N_CORES)]
    res = run_bass_kernel_spmd(nc, in_maps, list(range(N_CORES)))
    return unprep_out([res.results[i]["out"] for i in range(N_CORES)])


def kernel(**inputs) -> np.ndarray:
    return run_on_device(inputs, reps=1)
